# revision 11
# baseline (speedup 1.0000x reference)
"""Conv3d(32->64, k=3, pad=1) + BatchNorm(training) + LeakyReLU(0.2) on
(2, 32, 96, 96, 35), distributed over 8 TRN2 NeuronCores.

Strategy (v2):
  - Shard H (96 = 8 x 12 rows per core). Halo rows + spatial zero-padding are
    materialized host-side into a per-core tensor xs of shape (2,32,14,98,37).
  - Conv as implicit GEMM in bf16 (f32 PSUM accumulate): contraction
    K = 96 = C_in(32) x kd(3).  The SBUF "slab" for one input row holds 3
    partition-groups, group j pre-shifted by j elements along D.  Each of the
    9 (kh,kw) taps is one matmul whose rhs is a free-dim-shifted window of the
    slab; kd is folded into the contraction.  PSUM accumulates taps.
    The rhs streams an exact [12 w x 35 d] = 420-column window (2D free AP),
    skipping the 2 padded d columns (was 444 wide in v1).
  - M = C_out = 64 uses half the PE columns; two spatial w-tiles run as a
    column pair: tile A (w 0..47) -> psum[0:64], tile B (w 48..95) ->
    psum[64:128].  Per partition the 4 A (or B) tiles of one (b,h) block are
    contiguous in w, so pass-2 stores write 1680-element contiguous runs.
  - BatchNorm: bn_stats per evicted tile -> two-chunk bn_aggr (first 20
    blocks pre-aggregated during the pass-1 tail) -> (sum, sumsq) ->
    cross-core reduction -> scale/shift fused into one Prelu activation.
    Cross-core reduction: XOR all-to-all with remote_dma_broadcast (each core
    sends its 1KB partial to peer me^k's slot k; ~5us) instead of the ncfw
    AllReduce (~43us).  Set USE_REMOTE_AR=False to fall back.
  - Pass 2: per 2-row block, Prelu (scalar engine; every 3rd block on the
    vector engine as affine + max(0.2y, y)) -> bf16 -> store.  Output DRAM
    tensor is bf16; the host converts to f32.  Stores alternate between the
    SP and ACT HWDGE rings.
"""

import numpy as np
import ml_dtypes

import concourse.bacc as bacc
import concourse.bass as bass
import concourse.tile as tile
from concourse import mybir
from concourse.bass_utils import run_bass_kernel_spmd

N_CORES = 8
B, C_IN, C_OUT = 2, 32, 64
H, W, D = 96, 96, 35
HS = H // N_CORES          # 12 output rows per core
HR = HS + 2                # 14 input rows (halo)
WP, DP = W + 2, D + 2      # padded W / padded D for the host tensor
RW = DP                    # 37: slab row width per w-column (full padded D)
SLAB = WP * RW + 2 + 30    # slab row extent incl. group-shift + junk-read slack
WT = 12                    # w-tile width (8 uniform tiles)
EVF = WT * D               # 420 matmul/evict columns per tile
BLK = B * HS               # 24 (b,h) blocks per core
BLKCOLS = 4 * EVF          # 1680 conv-buffer columns per block per half
NREC = BLK * 4             # 96 bn_stats records per partition
SPLIT_BLK = 23             # stats chunk boundary (pre-aggregated early)
CNT1 = float(SPLIT_BLK * 4 * EVF)
CNT2 = float((BLK - SPLIT_BLK) * 4 * EVF)
N_TOT = float(B * H * W * D)        # 645120
CSTEP = HS * W * D                  # ys channel stride
EPS = 1e-5
NEG = 0.2
USE_REMOTE_AR = False

F32 = mybir.dt.float32
BF16 = mybir.dt.bfloat16
NP_BF16 = ml_dtypes.bfloat16

_CACHE = {}


def _build():
    nc = bacc.Bacc("TRN2", target_bir_lowering=False, debug=False,
                   num_devices=N_CORES)
    xs = nc.dram_tensor("xs", [B, C_IN, HR, WP, DP], BF16, kind="ExternalInput")
    wt = nc.dram_tensor("wt", [3, 3, 96, C_OUT], BF16, kind="ExternalInput")
    gm = nc.dram_tensor("gm", [128], F32, kind="ExternalInput")
    bt = nc.dram_tensor("bt", [128], F32, kind="ExternalInput")
    ys = nc.dram_tensor("ys", [B, C_OUT, HS, W, D], BF16, kind="ExternalOutput")

    xs_ap = xs.ap()
    ys_ap = ys.ap()

    from contextlib import ExitStack
    with tile.TileContext(nc) as tc:
        with tc.tile_pool(name="singles", bufs=1) as singles, \
             tc.tile_pool(name="dram", bufs=1, space="DRAM") as dramp:
            phase1 = ExitStack()
            slabp = phase1.enter_context(tc.tile_pool(name="slab", bufs=4))
            psump = phase1.enter_context(
                tc.tile_pool(name="psum", bufs=4, space="PSUM"))

            # ---- one-time loads ----
            wtile = singles.tile([96, 9, C_OUT], BF16)
            nc.sync.dma_start(
                out=wtile,
                in_=wt.ap().rearrange("kh kw p o -> p (kh kw) o"))
            gmt = singles.tile([128, 1], F32)
            btt = singles.tile([128, 1], F32)

            cb = singles.tile([128, BLK * BLKCOLS], BF16)   # conv results
            st = singles.tile([128, NREC * 6], F32)          # bn_stats records

            sq1 = singles.tile([128, 2], F32)
            t1a = singles.tile([128, 1], F32)

            # ---- pass 1: conv + stats ----
            # slab group tiles hold 2 input rows (rows 2g, 2g+1)
            for b in range(B):
                groups = {}
                for h in range(HS):
                    for r in (h, h + 1, h + 2):
                        g = r // 2
                        if g not in groups:
                            gt = slabp.tile([96, 2, SLAB], BF16, tag="slab")
                            for j in range(3):
                                # group j holds the full row shifted by (2-j);
                                # middle copy rides the ACT ring
                                eng = nc.scalar if j == 1 else nc.sync
                                eng.dma_start(
                                    out=gt[32 * j:32 * (j + 1), :,
                                           2 - j:2 - j + WP * RW],
                                    in_=xs_ap[b, :, 2 * g:2 * g + 2, :, :].rearrange(
                                        "p r w d -> p r (w d)"))
                            groups[g] = gt
                    blk = b * HS + h
                    for k in range(4):
                        w0a = k * WT               # tile A: w 0..47
                        w0b = (k + 4) * WT         # tile B: w 48..95
                        ps = psump.tile([128, EVF], F32, tag="ps")
                        for kh in range(3):
                            r = h + kh
                            gt = groups[r // 2]
                            rs = r % 2
                            for kw in range(3):
                                q = kh * 3 + kw
                                first, last = q == 0, q == 8
                                oa = (w0a + kw) * RW
                                ob = (w0b + kw) * RW
                                rhs_a = gt[:, rs, oa:oa + WT * RW].rearrange(
                                    "p (w d) -> p w d", d=RW)[:, :, 2:2 + D]
                                rhs_b = gt[:, rs, ob:ob + WT * RW].rearrange(
                                    "p (w d) -> p w d", d=RW)[:, :, 2:2 + D]
                                nc.tensor.matmul(
                                    ps[0:64, :],
                                    lhsT=wtile[:, q, :],
                                    rhs=rhs_a,
                                    start=first, stop=last)
                                nc.tensor.matmul(
                                    ps[64:128, :],
                                    lhsT=wtile[:, q, :],
                                    rhs=rhs_b,
                                    start=first, stop=last)
                        # evict + stats (both contiguous [128, 420])
                        col = blk * BLKCOLS + k * EVF
                        rec = (blk * 4 + k) * 6
                        nc.scalar.copy(out=cb[:, col:col + EVF], in_=ps)
                        nc.vector.bn_stats(out=st[:, rec:rec + 6],
                                           in_=cb[:, col:col + EVF])
                    if blk == SPLIT_BLK - 1:
                        # pre-aggregate stats of the first SPLIT_BLK blocks;
                        # runs during the pass-1 tail, off the critical path
                        mv1 = singles.tile([128, 2], F32)
                        nc.vector.bn_aggr(
                            out=mv1,
                            in_=st[:, 0:SPLIT_BLK * 24].rearrange(
                                "p (r s) -> p r s", s=6))
                        nc.vector.tensor_scalar_mul(sq1[:, 0:1], mv1[:, 0:1],
                                                    CNT1)
                        nc.vector.tensor_mul(t1a, mv1[:, 0:1], mv1[:, 0:1])
                        nc.vector.tensor_add(t1a, t1a, mv1[:, 1:2])
                        nc.vector.tensor_scalar_mul(sq1[:, 1:2], t1a, CNT1)

            phase1.close()

            # ---- stats: tail chunk + cross-core reduction ----
            # gamma/beta loads deferred here so they don't delay the first
            # slab DMAs on the ACT ring at startup
            nc.scalar.dma_start(out=gmt,
                                in_=gm.ap().rearrange("(p o) -> p o", o=1))
            nc.scalar.dma_start(out=btt,
                                in_=bt.ap().rearrange("(p o) -> p o", o=1))
            mv2 = singles.tile([128, 2], F32)
            nc.vector.bn_aggr(
                out=mv2,
                in_=st[:, SPLIT_BLK * 24:].rearrange("p (r s) -> p r s", s=6))
            sq = singles.tile([128, 2], F32)
            t2a = singles.tile([128, 1], F32)
            nc.vector.tensor_scalar_mul(sq[:, 0:1], mv2[:, 0:1], CNT2)
            nc.vector.tensor_mul(t2a, mv2[:, 0:1], mv2[:, 0:1])
            nc.vector.tensor_add(t2a, t2a, mv2[:, 1:2])
            nc.vector.tensor_scalar_mul(sq[:, 1:2], t2a, CNT2)
            nc.vector.tensor_add(sq, sq, sq1)

            tot = singles.tile([128, 2], F32)
            if USE_REMOTE_AR:
                # XOR all-to-all: core me sends sq to peer me^k's slot k.
                # Receiver slot k thus holds peer me^k's partial.  2 sem incs
                # per arrival (16 lanes / 8 dest slots), 7 arrivals -> 14.
                rsem = nc.alloc_semaphore("ar_rsem")
                lsem = nc.alloc_semaphore("ar_lsem")
                rx = singles.tile([128, 16], F32)
                with tc.tile_critical(name="remote_ar"):
                    for k in range(1, 8):
                        nc.gpsimd.remote_dma_broadcast(
                            rx[:, 2 * k:2 * k + 2], sq[:, :], rsem, lsem,
                            rdests=[(0, j) if j == k else None
                                    for j in range(8)])
                    nc.gpsimd.trigger_dma(count=None)
                    nc.gpsimd.wait_ge(rsem, 14)
                nc.vector.tensor_add(tot, sq, rx[:, 2:4])
                for k in range(2, 8):
                    nc.vector.tensor_add(tot, tot, rx[:, 2 * k:2 * k + 2])
            else:
                # AllGather instead of AllReduce: one ncfw phase instead of
                # two (RS+AG), halving the latency floor for this 1KB
                # latency-bound payload; the 8-way sum is 7 cheap vector adds.
                cc_in = dramp.tile([128, 2], F32)
                cc_out = dramp.tile([N_CORES, 128, 2], F32)
                nc.sync.dma_start(out=cc_in[:, :], in_=sq)
                nc.gpsimd.collective_compute(
                    "AllGather", mybir.AluOpType.bypass,
                    replica_groups=[list(range(N_CORES))],
                    ins=[cc_in[:, :].opt()], outs=[cc_out[:, :, :].opt()])
                rxg = singles.tile([128, 2 * N_CORES], F32)
                nc.sync.dma_start(
                    out=rxg[:, :].rearrange("p (r c) -> p r c", c=2),
                    in_=cc_out[:, :, :].rearrange("r p c -> p r c"))
                nc.vector.tensor_add(tot, rxg[:, 0:2], rxg[:, 2:4])
                for k in range(2, N_CORES):
                    nc.vector.tensor_add(tot, tot, rxg[:, 2 * k:2 * k + 2])

            # fold partition halves: tot2[p] = tot[p] + tot[p^64]
            sw = singles.tile([128, 2], F32)
            nc.sync.dma_start(out=sw[0:64, :], in_=tot[64:128, :])
            nc.scalar.dma_start(out=sw[64:128, :], in_=tot[0:64, :])
            tot2 = singles.tile([128, 2], F32)
            nc.vector.tensor_add(tot2, tot, sw)

            m_g = singles.tile([128, 1], F32)
            qn = singles.tile([128, 1], F32)
            var = singles.tile([128, 1], F32)
            sd = singles.tile([128, 1], F32)
            s_all = singles.tile([128, 1], F32)
            t_all = singles.tile([128, 1], F32)
            nc.vector.tensor_scalar_mul(m_g, tot2[:, 0:1], 1.0 / N_TOT)
            nc.vector.tensor_scalar_mul(qn, tot2[:, 1:2], 1.0 / N_TOT)
            nc.vector.tensor_mul(var, m_g, m_g)
            nc.vector.tensor_sub(var, qn, var)
            epst = singles.tile([128, 1], F32)
            nc.vector.memset(epst, EPS)
            nc.scalar.activation(out=sd, in_=var,
                                 func=mybir.ActivationFunctionType.Sqrt,
                                 bias=epst)
            nc.vector.reciprocal(out=sd, in_=sd)
            nc.vector.tensor_mul(s_all, sd, gmt)    # s = gamma * rsqrt(var+eps)
            nc.vector.tensor_mul(t_all, m_g, s_all)
            nc.vector.tensor_sub(t_all, btt, t_all)  # t = beta - mean * s

            # ---- pass 2: normalize + LeakyReLU + writeback (bf16) ----
            stgp = phase1.enter_context(tc.tile_pool(name="stg", bufs=3))
            for pi, blk in enumerate(range(0, BLK, 2)):
                b_, h_ = divmod(blk, HS)
                stg = stgp.tile([128, 2 * BLKCOLS], BF16, tag="stg")
                src = cb[:, blk * BLKCOLS:(blk + 2) * BLKCOLS]
                if pi % 2 == 1:
                    # vector path: y = s*x + t; leaky = max(0.2*y, y)
                    yt = stgp.tile([128, 2 * BLKCOLS], BF16, tag="yt")
                    nc.vector.tensor_scalar(yt, src, s_all, t_all,
                                            mybir.AluOpType.mult,
                                            mybir.AluOpType.add)
                    nc.vector.scalar_tensor_tensor(stg, yt, NEG, yt,
                                                   mybir.AluOpType.mult,
                                                   mybir.AluOpType.max)
                else:
                    nc.scalar.activation(
                        out=stg, in_=src,
                        func=mybir.ActivationFunctionType.Prelu,
                        bias=t_all, scale=s_all, alpha=NEG)
                base_off = ys_ap.offset + b_ * (C_OUT * CSTEP) + h_ * (W * D)
                # per partition: 2 h-rows x 1680 contiguous output elements
                dst_a = bass.AP(
                    tensor=ys_ap.tensor, offset=base_off,
                    ap=[[CSTEP, C_OUT], [W * D, 2], [1, BLKCOLS]])
                dst_b = bass.AP(
                    tensor=ys_ap.tensor, offset=base_off + BLKCOLS,
                    ap=[[CSTEP, C_OUT], [W * D, 2], [1, BLKCOLS]])
                src_a = stg[0:64, :].rearrange("p (r x) -> p r x", r=2)
                src_b = stg[64:128, :].rearrange("p (r x) -> p r x", r=2)
                nc.sync.dma_start(out=dst_a, in_=src_a)
                nc.scalar.dma_start(out=dst_b, in_=src_b)

            phase1.close()
    nc.finalize()
    return nc


def _get_nc():
    if "nc" not in _CACHE:
        _CACHE["nc"] = _build()
    return _CACHE["nc"]


def _prep(x, w, gamma, beta):
    xpad = np.zeros((B, C_IN, H + 2, WP, DP), dtype=np.float32)
    xpad[:, :, 1:H + 1, 1:W + 1, 1:D + 1] = x
    wt = np.ascontiguousarray(
        np.asarray(w, dtype=np.float32).transpose(2, 3, 4, 1, 0).reshape(
            3, 3, 96, C_OUT)).astype(NP_BF16)
    gm2 = np.ascontiguousarray(
        np.concatenate([np.asarray(gamma, dtype=np.float32)] * 2))
    bt2 = np.ascontiguousarray(
        np.concatenate([np.asarray(beta, dtype=np.float32)] * 2))
    in_maps = []
    for c in range(N_CORES):
        xsl = np.ascontiguousarray(
            xpad[:, :, c * HS:c * HS + HR, :, :]).astype(NP_BF16)
        in_maps.append({"xs": xsl, "wt": wt, "gm": gm2, "bt": bt2})
    return in_maps


def kernel(x, w, b, gamma, beta):
    nc = _get_nc()
    in_maps = _prep(np.asarray(x, dtype=np.float32), w, gamma, beta)
    res = run_bass_kernel_spmd(nc, in_maps, core_ids=list(range(N_CORES)))
    out = np.concatenate([res.results[c]["ys"] for c in range(N_CORES)], axis=2)
    return out.astype(np.float32)
